# revision 81
# baseline (speedup 1.0000x reference)
"""AlignmentModule (text/feats conv stacks -> L2 distance -> log_softmax + beta-binomial
prior) on 8 Trainium2 NeuronCores, data-parallel over batch (2 batches per core).

Layout strategy: everything runs channels-on-partitions. Host pre-transposes the
activations ([B,T,C] -> [B,C,T]) and conv weights (per-tap [C_in, C_out] = lhsT), so
every conv layer is a chain of PE matmuls accumulating over (tap, C_in-chunk) into
PSUM, and the distance cross-term consumes the conv outputs directly. ||f||^2 / ||t||^2
are folded into the same PSUM accumulation via an augmented K=2 matmul (ones/f2 rows
against t2/ones rows). The log-softmax tail is sqrt -> exp(fused row-sum) -> ln ->
one fused scalar_tensor_tensor with the prior; the stable-softmax shift is dist[:,0]
(any row element works: it upper-bounds the row min, so the shifted sum is >= 1 and
the ACT Ln table stays in its accurate range).

Schedule notes: batch 1's convs and score matmuls are interleaved with batch 0's
softmax phase; ACT activations are batched by table set (all Sqrt per batch, then
all Exp/Ln) so the HW pays 4 ACT table loads per core instead of 2 per f-tile
(~2.7us each). All conv PSUM evictions run on DVE so the ACT queue never convoys.

Matmuls use float32r (TF32-like rounded fp32): 4x the throughput of plain fp32 on the
PE at ~1.5e-4 relative error. The prior is fp16 and dist tiles bf16; total error vs
the fp32 reference is ~4.3e-4 of output absmax (dominated by fp16 prior rounding).
"""

import numpy as np

import concourse.bass as bass
import concourse.mybir as mybir
from concourse.tile import TileContext

F32 = mybir.dt.float32
F32R = mybir.dt.float32r
AF = mybir.ActivationFunctionType
OP = mybir.AluOpType
AX = mybir.AxisListType

B, T_TEXT, T_FEATS, ADIM, ODIM = 16, 512, 2048, 256, 80
N_CORES = 8
B_LOC = B // N_CORES  # 2 batches per core


def _split_excess_waits(nc, limit=1):
    """walrus CoreV3 CTRL codegen here rejects >1 sync-wait per instruction.
    Hoist excess waits onto preceding NOPs on the same engine."""
    ctr = 0
    for f in nc.m.functions:
        for bb in f.blocks:
            insts = bb.instructions
            idx = 0
            while idx < len(insts):
                ins = insts[idx]
                si = ins.sync_info
                if si is not None and len(si.on_wait) > limit:
                    waits = list(si.on_wait)
                    extra, keep = waits[:-limit], waits[-limit:]
                    si.on_wait = keep
                    pos = idx
                    for j in range(0, len(extra), limit):
                        nop = mybir.InstNoOp(name=f"waitsplit_{ctr}", ins=[], outs=[])
                        ctr += 1
                        nop.engine = ins.engine
                        nop.sync_info = mybir.SyncInfo(
                            on_wait=extra[j : j + limit], on_update=[]
                        )
                        insts.insert(pos, nop)
                        pos += 1
                        idx += 1
                idx += 1
    return ctr


def _beta_binomial_prior():
    """prior[f, t] = betabinom_logpmf(k=t; n=T_TEXT, a=f+1, b=T_FEATS-f), fp64 host."""
    from scipy.special import gammaln

    T, N = T_FEATS, T_TEXT
    a = np.arange(1, T + 1, dtype=np.float64)[:, None]  # alpha, [T,1]
    b = (T - np.arange(1, T + 1, dtype=np.float64) + 1.0)[:, None]  # beta, [T,1]
    k = np.arange(N, dtype=np.float64)[None, :]  # [1,N]
    n = float(N)

    def betaln(x, y):
        return gammaln(x) + gammaln(y) - gammaln(x + y)

    logp = (
        gammaln(n + 1.0)
        - gammaln(k + 1.0)
        - gammaln(n - k + 1.0)
        + betaln(k + a, n - k + b)
        - betaln(a, b)
    )
    return logp.astype(np.float32)  # [T_FEATS, T_TEXT]


def _build_nc():
    nc = bass.Bass(name="alignment")

    TT, TF = T_TEXT, T_FEATS
    NT = TF // 512  # 4 feats T-chunks of 512
    NF = TF // 128  # 16 f-tiles per batch

    # --- DRAM I/O (per core) ---
    textT = nc.dram_tensor("textT", [B_LOC, ADIM, TT], F32R, kind="ExternalInput")
    featsT = nc.dram_tensor("featsT", [B_LOC, ODIM, TF], F32R, kind="ExternalInput")
    tw1 = nc.dram_tensor("tw1", [3, ADIM, 2, 128], F32R, kind="ExternalInput")
    tw2 = nc.dram_tensor("tw2", [ADIM, 2, 128], F32R, kind="ExternalInput")
    fw1 = nc.dram_tensor("fw1", [3, ODIM, 2, 128], F32R, kind="ExternalInput")
    fw2 = nc.dram_tensor("fw2", [3, ADIM, 2, 128], F32R, kind="ExternalInput")
    fw3 = nc.dram_tensor("fw3", [ADIM, 2, 128], F32R, kind="ExternalInput")
    tb1 = nc.dram_tensor("tb1", [2, 128], F32, kind="ExternalInput")
    tb2s = nc.dram_tensor("tb2s", [2, 128], F32, kind="ExternalInput")  # t_b2
    fb1 = nc.dram_tensor("fb1", [2, 128], F32, kind="ExternalInput")
    fb2 = nc.dram_tensor("fb2", [2, 128], F32, kind="ExternalInput")
    fb3 = nc.dram_tensor("fb3", [2, 128], F32, kind="ExternalInput")
    priorD = nc.dram_tensor("prior", [TF, TT], mybir.dt.float16, kind="ExternalInput")
    outD = nc.dram_tensor("out", [B_LOC, TF, TT], F32, kind="ExternalOutput")

    with TileContext(nc) as tc:
        with (
            tc.tile_pool(name="const", bufs=1) as const,
            tc.tile_pool(name="wpool", bufs=1) as wpool,
            tc.tile_pool(name="txp", bufs=1) as txp,
            tc.tile_pool(name="txp2", bufs=2) as txp2,
            tc.tile_pool(name="ftp", bufs=1) as ftp,
            tc.tile_pool(name="ftp2", bufs=2) as ftp2,
            tc.tile_pool(name="scoreD", bufs=16) as scoreD,
            tc.tile_pool(name="scoreE", bufs=2) as scoreE,
            tc.tile_pool(name="scoreO", bufs=8) as scoreO,
            tc.tile_pool(name="small", bufs=8) as small,
            tc.tile_pool(name="smallM", bufs=16) as smallM,
            tc.tile_pool(name="f2t", bufs=2) as f2t,
            tc.tile_pool(name="pp", bufs=4, space="PSUM") as pp,
            tc.tile_pool(name="pps", bufs=3, space="PSUM") as pps,
            tc.tile_pool(name="ppn", bufs=1, space="PSUM") as ppn,
        ):
            F16 = mybir.dt.float16
            BF16 = mybir.dt.bfloat16

            # ---- batch-0 activations first so PE can start ASAP ----
            def load_tx0(b):
                tx0 = txp.tile([128, 2, TT + 2], F32R, tag="tx0")
                nc.vector.memset(tx0[:, :, 0:1].bitcast(F32), 0.0)
                nc.vector.memset(tx0[:, :, TT + 1 : TT + 2].bitcast(F32), 0.0)
                nc.gpsimd.dma_start(
                    out=tx0[:, :, 1 : TT + 1],
                    in_=textT[b].rearrange("(c p) t -> p c t", p=128),
                )
                return tx0

            def load_ft0(b):
                ft0 = ftp.tile([ODIM, TF + 2], F32R, tag="ft0")
                nc.vector.memset(ft0[:, 0:1].bitcast(F32), 0.0)
                nc.vector.memset(ft0[:, TF + 1 : TF + 2].bitcast(F32), 0.0)
                nc.gpsimd.dma_start(out=ft0[:, 1 : TF + 1], in_=featsT[b])
                return ft0

            tx0_0 = load_tx0(0)

            # ---- weights (text first so t-branch starts ASAP) ----
            tw1_sb = wpool.tile([128, 3, 2, 2, 128], F32R, tag="tw1")
            for k in range(3):
                nc.sync.dma_start(
                    out=tw1_sb[:, k],
                    in_=tw1[k].rearrange("(c p) m q -> p c m q", p=128),
                )
            tb1_sb = wpool.tile([128, 2], F32, tag="tb1")
            nc.sync.dma_start(out=tb1_sb[:], in_=tb1.rearrange("m q -> q m"))
            ft0_0 = load_ft0(0)
            tw2_sb = wpool.tile([128, 2, 2, 128], F32R, tag="tw2")
            nc.sync.dma_start(
                out=tw2_sb[:], in_=tw2.rearrange("(c p) m q -> p c m q", p=128)
            )
            tb2s_sb = wpool.tile([128, 2], F32, tag="tb2s")
            nc.sync.dma_start(out=tb2s_sb[:], in_=tb2s.rearrange("m q -> q m"))
            fw1_sb = wpool.tile([ODIM, 3, 2, 128], F32R, tag="fw1")
            for k in range(3):
                nc.sync.dma_start(out=fw1_sb[:, k], in_=fw1[k])
            fb1_sb = wpool.tile([128, 2], F32, tag="fb1")
            nc.sync.dma_start(out=fb1_sb[:], in_=fb1.rearrange("m q -> q m"))
            fw2_sb = wpool.tile([128, 3, 2, 2, 128], F32R, tag="fw2")
            for k in range(3):
                nc.sync.dma_start(
                    out=fw2_sb[:, k],
                    in_=fw2[k].rearrange("(c p) m q -> p c m q", p=128),
                )
            fb2_sb = wpool.tile([128, 2], F32, tag="fb2")
            nc.sync.dma_start(out=fb2_sb[:], in_=fb2.rearrange("m q -> q m"))
            fw3_sb = wpool.tile([128, 2, 2, 128], F32R, tag="fw3")
            nc.sync.dma_start(
                out=fw3_sb[:], in_=fw3.rearrange("(c p) m q -> p c m q", p=128)
            )
            fb3_sb = wpool.tile([128, 2], F32, tag="fb3")
            nc.sync.dma_start(out=fb3_sb[:], in_=fb3.rearrange("m q -> q m"))

            # ---- small constants ----
            prior_sb = const.tile([128, NF, TT], F16)
            eps_t = const.tile([128, 1], F32)
            nc.vector.memset(eps_t[:], 1e-12)
            ones_col = const.tile([128, 1], F32R)
            nc.vector.memset(ones_col[:].bitcast(F32), 1.0)
            ones_row = const.tile([1, TT], F32R)
            nc.vector.memset(ones_row[:].bitcast(F32), 1.0)
            # prior is only needed by the score tail; issued after conv inputs
            nc.sync.dma_start(
                out=prior_sb[:], in_=priorD.rearrange("(i p) t -> p i t", p=128)
            )

            def conv_steps(b, tx0, ft0, fused_score=None):
                """Yield one closure per PSUM-group of the conv phase."""
                tx1 = txp.tile([128, 2, TT + 2], F32R, tag="tx1")
                tx2m2 = txp2.tile([128, 2, TT], F32R, tag="tx2m2")
                txsq = txp.tile([128, 2, TT], F32R, tag="txsq")
                t2sb = txp2.tile([2, TT], F32R, tag="t2sb")
                ft1 = ftp.tile([128, 2, TF + 2], F32R, tag="ft1")
                ft2 = ftp.tile([128, 2, TF + 2], F32R, tag="ft2")
                ft3 = ftp2.tile([128, 2, TF], F32R, tag="ft3")
                ftsq = ftp.tile([128, 2, TF], F32R, tag="ftsq")
                f2aug = ftp2.tile([2, TF], F32R, tag="f2aug")

                yield ("tiles", (tx2m2, t2sb, ft3, f2aug))

                def pads():
                    nc.vector.memset(tx1[:, :, 0:1].bitcast(F32), 0.0)
                    nc.vector.memset(tx1[:, :, TT + 1 : TT + 2].bitcast(F32), 0.0)
                    nc.vector.memset(ft1[:, :, 0:1].bitcast(F32), 0.0)
                    nc.vector.memset(ft1[:, :, TF + 1 : TF + 2].bitcast(F32), 0.0)
                    nc.vector.memset(ft2[:, :, 0:1].bitcast(F32), 0.0)
                    nc.vector.memset(ft2[:, :, TF + 1 : TF + 2].bitcast(F32), 0.0)
                    nc.vector.memset(f2aug[0:1, :].bitcast(F32), 1.0)
                    nc.sync.dma_start(out=t2sb[1:2, :], in_=ones_row[:])

                yield pads

                def t1_step(m):
                    def f():
                        ps = pp.tile([128, TT], F32, tag="ps512")
                        first = True
                        for k in range(3):
                            for c in range(2):
                                nc.tensor.matmul(
                                    ps[:],
                                    tw1_sb[:, k, c, m, :],
                                    tx0[:, c, k : k + TT],
                                    start=first,
                                    stop=(k == 2 and c == 1),
                                )
                                first = False
                        nc.vector.tensor_scalar(
                            tx1[:, m, 1 : TT + 1], ps[:],
                            tb1_sb[:, m : m + 1], 0.0, OP.add, OP.max,
                        )
                    return f


                def t2_step(m):
                    def f():
                        ps = pp.tile([128, TT], F32, tag="ps512")
                        for c in range(2):
                            nc.tensor.matmul(
                                ps[:],
                                tw2_sb[:, c, m, :],
                                tx1[:, c, 1 : TT + 1],
                                start=(c == 0),
                                stop=(c == 1),
                            )
                        nc.vector.tensor_scalar(
                            tx2m2[:, m, :], ps[:],
                            tb2s_sb[:, m : m + 1], -2.0, OP.add, OP.mult,
                        )
                        nc.gpsimd.tensor_tensor(
                            txsq[:, m, :], tx2m2[:, m, :].bitcast(F32),
                            tx2m2[:, m, :].bitcast(F32), OP.mult,
                        )
                    return f


                def t2row_step():
                    psn = ppn.tile([1, TT], F32, tag="psn")
                    for c in range(2):
                        nc.tensor.matmul(
                            psn[:], ones_col[:], txsq[:, c, :],
                            start=(c == 0), stop=(c == 1),
                        )
                    nc.vector.tensor_scalar(t2sb[0:1, :], psn[:], 0.25, None, OP.mult)


                def f1_step(m, n):
                    def f():
                        ps = pp.tile([128, 512], F32, tag="ps512")
                        for k in range(3):
                            nc.tensor.matmul(
                                ps[:],
                                fw1_sb[:, k, m, :],
                                ft0[:, n * 512 + k : n * 512 + k + 512],
                                start=(k == 0),
                                stop=(k == 2),
                            )
                        nc.vector.tensor_scalar(
                            ft1[:, m, 1 + n * 512 : 1 + (n + 1) * 512],
                            ps[:], fb1_sb[:, m : m + 1], 0.0, OP.add, OP.max,
                        )
                    return f

                yield t1_step(0)
                yield t1_step(1)
                yield t2_step(0)
                yield t2_step(1)
                yield t2row_step
                for n in range(NT):
                    for m in range(2):
                        yield f1_step(m, n)

                def f2_step(m, n):
                    def f():
                        ps = pp.tile([128, 512], F32, tag="ps512")
                        first = True
                        for k in range(3):
                            for c in range(2):
                                nc.tensor.matmul(
                                    ps[:],
                                    fw2_sb[:, k, c, m, :],
                                    ft1[:, c, n * 512 + k : n * 512 + k + 512],
                                    start=first,
                                    stop=(k == 2 and c == 1),
                                )
                                first = False
                        nc.vector.tensor_scalar(
                            ft2[:, m, 1 + n * 512 : 1 + (n + 1) * 512],
                            ps[:], fb2_sb[:, m : m + 1], 0.0, OP.add, OP.max,
                        )
                    return f

                for n in range(NT):
                    for m in range(2):
                        yield f2_step(m, n)

                def f3_step(m, n):
                    def f():
                        ps = pp.tile([128, 512], F32, tag="ps512")
                        for c in range(2):
                            nc.tensor.matmul(
                                ps[:],
                                fw3_sb[:, c, m, :],
                                ft2[:, c, 1 + n * 512 : 1 + (n + 1) * 512],
                                start=(c == 0),
                                stop=(c == 1),
                            )
                        sl = slice(n * 512, (n + 1) * 512)
                        nc.vector.tensor_scalar(
                            ft3[:, m, sl], ps[:], fb3_sb[:, m : m + 1], None, OP.add
                        )
                        nc.gpsimd.tensor_tensor(
                            ftsq[:, m, sl], ft3[:, m, sl].bitcast(F32),
                            ft3[:, m, sl].bitcast(F32), OP.mult,
                        )
                    return f


                def f2row_step(n):
                    def f():
                        psn = ppn.tile([1, 512], F32, tag="psn")
                        for c in range(2):
                            nc.tensor.matmul(
                                psn[:],
                                ones_col[:],
                                ftsq[:, c, n * 512 : (n + 1) * 512],
                                start=(c == 0),
                                stop=(c == 1),
                            )
                        sl = slice(n * 512, (n + 1) * 512)
                        f2tmp = f2t.tile([1, 512], F32R, tag="f2tmp")
                        nc.vector.tensor_copy(f2tmp[0:1, :], psn[:])
                        nc.sync.dma_start(out=f2aug[1:2, sl], in_=f2tmp[0:1, :])
                    return f

                for n in range(NT):
                    yield f3_step(0, n)
                    yield f3_step(1, n)
                    if n >= 2:
                        yield f2row_step(n - 2)
                        if fused_score is not None:
                            for i in range(4 * (n - 2), 4 * (n - 1)):
                                yield ("fused", fused_score(n - 2, i))
                for n in (NT - 2, NT - 1):
                    yield f2row_step(n)
                    if fused_score is not None:
                        for i in range(4 * n, 4 * (n + 1)):
                            yield ("fused", fused_score(n, i))



            def score_mm_sqrt_step(b, i, tx2m2, t2sb, ft3, f2aug, dist_tiles):
                    def f():
                        fsl = slice(i * 128, (i + 1) * 128)
                        ps = pps.tile([128, TT], F32, tag="pscore")
                        nc.tensor.matmul(
                            ps[:], ft3[:, 0, fsl], tx2m2[:, 0, :], start=True, stop=False
                        )
                        nc.tensor.matmul(
                            ps[:], ft3[:, 1, fsl], tx2m2[:, 1, :], start=False, stop=False
                        )
                        nc.tensor.matmul(
                            ps[:], f2aug[:, fsl], t2sb[:], start=False, stop=True
                        )
                        # dist in bf16: 16 tiles stay live until the exp/ln phase.
                        # ACT table sets are batched: all Sqrt (one set) now, all
                        # Exp/Ln (one set) later -- 2 table loads per batch instead
                        # of 2 per tile (~2.7us each on HW).
                        dist = scoreD.tile([128, TT], BF16, tag="dist")
                        nc.scalar.activation(dist[:], ps[:], AF.Sqrt, bias=eps_t[:])
                        mmin = smallM.tile([128, 1], F32, tag="mmin")
                        # softmax shift: any row element works (sum >= 1 since some
                        # dist <= m; spread << 88 keeps exp finite). Copy is a
                        # filler function present in every ACT table set.
                        nc.scalar.copy(mmin[:], dist[:, 0:1])
                        dist_tiles[i] = (dist, mmin)
                    return f

            def score_softmax_step(b, i, dist_tiles):
                    def f():
                        fsl = slice(i * 128, (i + 1) * 128)
                        dist, mmin = dist_tiles[i]
                        e = scoreE.tile([128, TT], BF16, tag="e")
                        ssum = small.tile([128, 1], F32, tag="ssum")
                        nc.scalar.activation(
                            e[:], dist[:], AF.Exp, scale=-1.0, bias=mmin[:],
                            accum_out=ssum[:],
                        )
                        lns = small.tile([128, 1], F32, tag="lns")
                        nc.scalar.activation(lns[:], ssum[:], AF.Ln)
                        cc = small.tile([128, 1], F32, tag="cc")
                        nc.vector.tensor_sub(cc[:], lns[:], mmin[:])
                        outp = scoreO.tile([128, TT], F32, tag="outp")
                        nc.vector.scalar_tensor_tensor(
                            outp[:], prior_sb[:, i, :], cc[:], dist[:],
                            OP.subtract, OP.subtract,
                        )
                        nc.sync.dma_start(out=outD[b, fsl, :], in_=outp[:])
                    return f

            # both batches: conv phases with per-chunk fused matmul+sqrt tiles;
            # the exp/ln (softmax) phase of each batch runs after its convs so the
            # ACT table set switches only twice per batch.
            tiles0_box, dist0 = [], {}

            def fused0(n, i):
                tx2m2, t2sb, ft3, f2aug = tiles0_box[0]
                return score_mm_sqrt_step(0, i, tx2m2, t2sb, ft3, f2aug, dist0)

            conv0 = conv_steps(0, tx0_0, ft0_0, fused_score=fused0)
            for step in conv0:
                if isinstance(step, tuple):
                    if step[0] == "tiles":
                        tiles0_box.append(step[1])
                        continue
                    step = step[1]
                step()

            tx0_1 = load_tx0(1)
            ft0_1 = load_ft0(1)
            tiles1_box, dist1 = [], {}

            def fused1(n, i):
                tx2m2, t2sb, ft3, f2aug = tiles1_box[0]
                return score_mm_sqrt_step(1, i, tx2m2, t2sb, ft3, f2aug, dist1)

            conv1 = conv_steps(1, tx0_1, ft0_1, fused_score=fused1)
            sm0 = [score_softmax_step(0, i, dist0) for i in range(NF)]
            si = 0
            for step in conv1:
                if isinstance(step, tuple):
                    if step[0] == "tiles":
                        tiles1_box.append(step[1])
                        continue
                    # batch-1 sqrt begins: emit batch-0's whole exp/ln block
                    # here so the ACT table set switches exactly once
                    while si < len(sm0):
                        sm0[si]()
                        si += 1
                    step = step[1]
                step()
            while si < len(sm0):
                sm0[si]()
                si += 1
            for i in range(NF):
                score_softmax_step(1, i, dist1)()

    _split_excess_waits(nc)
    return nc


_NC = None


def _get_nc():
    global _NC
    if _NC is None:
        _NC = _build_nc()
    return _NC


def _prep_inputs(text, feats, t_w1, t_b1, t_w2, t_b2, f_w1, f_b1, f_w2, f_b2, f_w3, f_b3):
    c = np.ascontiguousarray
    f4 = np.float32
    textT = c(text.astype(f4).transpose(0, 2, 1))  # [B, ADIM, TT]
    featsT = c(feats.astype(f4).transpose(0, 2, 1))  # [B, ODIM, TF]
    shared = {
        "tw1": c(t_w1.astype(f4).transpose(2, 1, 0)).reshape(3, ADIM, 2, 128),
        "tw2": c(t_w2.astype(f4)[:, :, 0].T).reshape(ADIM, 2, 128),
        "fw1": c(f_w1.astype(f4).transpose(2, 1, 0)).reshape(3, ODIM, 2, 128),
        "fw2": c(f_w2.astype(f4).transpose(2, 1, 0)).reshape(3, ADIM, 2, 128),
        "fw3": c(f_w3.astype(f4)[:, :, 0].T).reshape(ADIM, 2, 128),
        "tb1": c(t_b1.astype(f4)).reshape(2, 128),
        "tb2s": c(t_b2.astype(f4)).reshape(2, 128),
        "fb1": c(f_b1.astype(f4)).reshape(2, 128),
        "fb2": c(f_b2.astype(f4)).reshape(2, 128),
        "fb3": c(f_b3.astype(f4)).reshape(2, 128),
        "prior": _beta_binomial_prior().astype(np.float16),
    }
    in_maps = []
    for core in range(N_CORES):
        m = dict(shared)
        m["textT"] = c(textT[core * B_LOC : (core + 1) * B_LOC])
        m["featsT"] = c(featsT[core * B_LOC : (core + 1) * B_LOC])
        in_maps.append(m)
    return in_maps


_CALLABLE = None


def _build_callable():
    """Compile once; return a function(in_maps) -> list of per-core output dicts.
    Mirrors concourse.bass2jax.run_bass_via_pjrt but keeps the jitted executable
    (and the NEFF behind it) alive across kernel() calls."""
    import jax
    import jax.numpy as jnp
    from jax.sharding import Mesh, NamedSharding, PartitionSpec
    from jax.experimental.shard_map import shard_map
    from concourse.bass2jax import (
        _bass_exec_p,
        install_neuronx_cc_hook,
        partition_id_tensor,
    )

    nc = _get_nc()
    install_neuronx_cc_hook()
    partition_name = nc.partition_id_tensor.name if nc.partition_id_tensor else None
    in_names, out_names, out_avals, zero_shapes = [], [], [], []
    for alloc in nc.m.functions[0].allocations:
        if not isinstance(alloc, mybir.MemoryLocationSet):
            continue
        name = alloc.memorylocations[0].name
        if alloc.kind == "ExternalInput":
            if name != partition_name:
                in_names.append(name)
        elif alloc.kind == "ExternalOutput":
            shape = tuple(alloc.tensor_shape)
            dtype = mybir.dt.np(alloc.dtype)
            out_names.append(name)
            out_avals.append(jax.core.ShapedArray(shape, dtype))
            zero_shapes.append(((N_CORES * shape[0],) + shape[1:], dtype))
    n_params = len(in_names)
    n_outs = len(out_avals)
    all_in_names = list(in_names) + out_names
    if partition_name is not None:
        all_in_names.append(partition_name)
    donate = tuple(range(n_params, n_params + n_outs))

    def _body(*args):
        operands = list(args)
        if partition_name is not None:
            operands.append(partition_id_tensor())
        outs = _bass_exec_p.bind(
            *operands,
            out_avals=tuple(out_avals),
            in_names=tuple(all_in_names),
            out_names=tuple(out_names),
            lowering_input_output_aliases=(),
            sim_require_finite=True,
            sim_require_nnan=True,
            nc=nc,
        )
        return tuple(outs)

    devices = jax.devices()[:N_CORES]
    mesh = Mesh(np.asarray(devices), ("core",))
    fn = jax.jit(
        shard_map(
            _body,
            mesh=mesh,
            in_specs=(PartitionSpec("core"),) * (n_params + n_outs),
            out_specs=(PartitionSpec("core"),) * n_outs,
            check_rep=False,
        ),
        donate_argnums=donate,
        keep_unused=True,
    )
    sharding = NamedSharding(mesh, PartitionSpec("core"))
    zfn = jax.jit(
        lambda: tuple(jnp.zeros(s, d) for s, d in zero_shapes),
        out_shardings=tuple(sharding for _ in zero_shapes),
    )

    def call(in_maps):
        concat_in = [
            np.concatenate([np.asarray(in_maps[c][n]) for c in range(N_CORES)], axis=0)
            for n in in_names
        ]
        out_arrs = fn(*concat_in, *zfn())
        return [
            {
                name: np.asarray(out_arrs[i]).reshape(
                    N_CORES, *out_avals[i].shape
                )[c]
                for i, name in enumerate(out_names)
            }
            for c in range(N_CORES)
        ]

    return call


def _run(inputs, **kw):
    global _CALLABLE
    in_maps = _prep_inputs(
        inputs["text"], inputs["feats"],
        inputs["t_w1"], inputs["t_b1"], inputs["t_w2"], inputs["t_b2"],
        inputs["f_w1"], inputs["f_b1"], inputs["f_w2"], inputs["f_b2"],
        inputs["f_w3"], inputs["f_b3"],
    )
    results = None
    if _CALLABLE is not False:
        # Fast path: cached PJRT executable (axon / bass2jax). Falls back to
        # the stock run_bass_kernel_spmd on any environment mismatch.
        try:
            if _CALLABLE is None:
                from concourse._compat import axon_active

                if not axon_active():
                    raise RuntimeError("axon not active; use native path")
                _CALLABLE = _build_callable()
            results = _CALLABLE(in_maps)
        except Exception:
            _CALLABLE = False
            results = None
    if results is None:
        from concourse.bass_utils import run_bass_kernel_spmd

        results = run_bass_kernel_spmd(
            _get_nc(), in_maps, core_ids=list(range(N_CORES))
        ).results
    out = np.concatenate([r["out"] for r in results], axis=0)
    return out, results


def kernel(**inputs) -> np.ndarray:
    out, _ = _run(inputs)
    return out
